# revision 17
# baseline (speedup 1.0000x reference)
"""Trainium2 Bass kernel for AAGNetSegmentor (1024-face graph transformer + all-pairs
instance head), SPMD across 8 NeuronCores.

Sharding: faces (N=1024) split into 8 row-blocks of 128. Backbone is sequence-parallel
with per-layer AllGather of K/V (bf16, head-padded to 32-partition alignment). The
N x N pair head is computed as per-core COLUMN blocks of the symmetric output (each
core computes [all j, own i] via row symmetry). Each j-chunk needs only ONE variant
(upper or lower), selected per-core at runtime through register-offset APs driven by
a host-provided table; the diagonal chunk is completed with its own transpose plus
triangular masks.

kernel(**inputs) takes the full unsharded inputs and returns
(seg_out [1024,25], inst_matrix [1024,1024], bottom_out [1024]) as float32.
"""

import numpy as np

# problem constants
N = 1024
NODE_ATTR = 10
GRID = 7
E = 128
NH = 8
HD = 16
L = 3
MLP_H = 256
HEAD = 64
NCLS = 25
LN_EPS = 1e-5

NCORES = 8
NL = N // NCORES  # 128

# pair-head engine assignment (h-axis is host-sorted: positive w2 first)
N_ACT_GROUPS = 2  # groups of 8 h's relu'd on ACT (needs 8*N_ACT_GROUPS <= npos)

_GRAPH_CACHE = {}


def _build(npos):
    """npos: number of positive-w2 h's (h-axis pre-sorted: 0..npos-1 positive)."""
    import concourse.bass as bass
    import concourse.bacc as bacc
    import concourse.tile as tile
    import concourse.mybir as mybir
    from concourse.masks import make_identity
    from contextlib import ExitStack

    F32 = mybir.dt.float32
    BF16 = mybir.dt.bfloat16
    I32 = mybir.dt.int32
    Alu = mybir.AluOpType
    Act = mybir.ActivationFunctionType

    nc = bacc.Bacc("TRN2", target_bir_lowering=False)

    # ---------------- external I/O ----------------
    blob_cols, blob_spec = _blob_spec()
    P = {}

    def par(name, shape, dtype=F32):
        P[name] = nc.declare_dram_parameter(name, list(shape), dtype, isOutput=False)
        return P[name]

    par("blob", [128, blob_cols])
    par("rows14", [14, E])          # broadcast rows (ln/out_b/mlp_b2/w2/badd)
    par("masks3", [N, 3 * NL])      # per chunk: [mR | mM | mB]

    segT_out = nc.declare_dram_parameter("segT_out", [NCLS, NL], F32, isOutput=True)
    bot_out = nc.declare_dram_parameter("bot_out", [1, NL], F32, isOutput=True)
    instT_out = nc.declare_dram_parameter("instT_out", [N, NL], F32, isOutput=True)

    with tile.TileContext(nc) as tc, ExitStack() as top:
        wpool = top.enter_context(tc.tile_pool(name="weights", bufs=1))
        cpool = top.enter_context(tc.tile_pool(name="consts", bufs=1))
        persist = top.enter_context(tc.tile_pool(name="persist", bufs=1))
        dram = top.enter_context(tc.tile_pool(name="dram", bufs=1, space="DRAM"))

        # ---------------- params: one blob DMA; weights = slices ----------------
        blob = wpool.tile([128, blob_cols], F32)
        nc.sync.dma_start(blob, P["blob"][:, :])

        def W(name, l=None, j=None):
            key = name if l is None else (f"{name}_{l}" if j is None
                                          else f"{name}_{l}_{j}")
            p, c0, c1 = blob_spec[key]
            return blob[0:p, c0:c1]

        # broadcast tiles from rows14: one batched broadcast DMA
        rows_bc = cpool.tile([128, 14 * E], F32)

        def part_bcast(ap_slice, parts=128):
            return bass.AP(tensor=ap_slice.tensor, offset=ap_slice.offset,
                           ap=[[0, parts]] + [list(p) for p in ap_slice.ap])

        nc.sync.dma_start(rows_bc.rearrange("p (r f) -> p r f", r=14),
                          part_bcast(P["rows14"][:, :]))
        ROWS = ["out_b_0", "out_b_1", "out_b_2", "ln_w_0", "ln_w_1", "ln_w_2",
                "ln_b_0", "ln_b_1", "ln_b_2", "mlp_b2_0", "mlp_b2_1", "mlp_b2_2",
                "w2row", "baddrow"]

        def BC(key):
            r = ROWS.index(key)
            return rows_bc[:, E * r:E * (r + 1)]

        ident_f = cpool.tile([128, 128], F32)
        make_identity(nc, ident_f)
        ident_b = cpool.tile([128, 128], BF16)
        make_identity(nc, ident_b)
        eps_col = cpool.tile([NL, 1], F32)
        nc.vector.memset(eps_col, LN_EPS)

        # ---------------- embed ----------------
        feat_nat = persist.tile([NL, E], F32)
        featT = persist.tile([E, NL], F32)

        with tc.tile_pool(name="emb_ps", bufs=2, space="PSUM") as eps_pool:
            ft_ps = eps_pool.tile([E, NL], F32)
            nc.tensor.matmul(ft_ps, W("emb_wT"), W("xT"), start=True, stop=True)
            nc.scalar.activation(featT, ft_ps, Act.Identity, bias=W("emb_b"))
            fn_ps = eps_pool.tile([NL, E], F32)
            nc.tensor.transpose(fn_ps, featT, ident_f)
            nc.vector.tensor_copy(feat_nat, fn_ps)

        # ---------------- transformer layers ----------------
        for l in range(L):
            with ExitStack() as lyr:
                ps_m = lyr.enter_context(
                    tc.tile_pool(name=f"ps_m_{l}", bufs=2, space="PSUM"))
                ps_s = lyr.enter_context(
                    tc.tile_pool(name=f"ps_s_{l}", bufs=1, space="PSUM"))
                ps_t = lyr.enter_context(
                    tc.tile_pool(name=f"ps_t_{l}", bufs=2, space="PSUM"))
                ps_o = lyr.enter_context(
                    tc.tile_pool(name=f"ps_o_{l}", bufs=1, space="PSUM"))
                sb = lyr.enter_context(tc.tile_pool(name=f"sb_{l}", bufs=2))
                sbT = lyr.enter_context(tc.tile_pool(name=f"sbT_{l}", bufs=3))

                # qkv; q/k head-padded: 3 tiles of [96, NL] each
                qp = []
                kp_loc = []
                for t3 in range(3):
                    q_ps = ps_m.tile([96, NL], F32, name=f"q_ps_{l}_{t3}", tag="m")
                    nc.tensor.matmul(q_ps, W("ipT", l)[:, 96 * t3:96 * (t3 + 1)],
                                     featT, start=True, stop=True)
                    qt = sb.tile([96, NL], BF16, name=f"qp_{l}_{t3}", bufs=2)
                    nc.scalar.activation(qt, q_ps, Act.Identity,
                                         bias=W("ipb", l, t3))
                    qp.append(qt)
                    k_ps = ps_m.tile([96, NL], F32, name=f"k_ps_{l}_{t3}", tag="m")
                    nc.tensor.matmul(
                        k_ps, W("ipT", l)[:, 288 + 96 * t3:288 + 96 * (t3 + 1)],
                        featT, start=True, stop=True)
                    kt = sb.tile([96, NL], BF16, name=f"kp_{l}_{t3}", bufs=2)
                    nc.scalar.activation(kt, k_ps, Act.Identity,
                                         bias=W("ipb", l, 3 + t3))
                    kp_loc.append(kt)
                v_ps = ps_m.tile([E, NL], F32, name=f"v_ps_{l}", tag="m")
                nc.tensor.matmul(v_ps, W("ipT", l)[:, 576:704], featT,
                                 start=True, stop=True)
                vT = sb.tile([E, NL], F32, name=f"vT_{l}", bufs=2)
                nc.scalar.activation(vT, v_ps, Act.Identity, bias=W("ipb", l, 6))
                vn_ps = ps_m.tile([NL, E], F32, name=f"vn_ps_{l}", tag="m")
                nc.tensor.transpose(vn_ps, vT, ident_f)
                v_loc_bf = sb.tile([NL, E], BF16, name=f"vlocbf_{l}")
                nc.vector.tensor_copy(v_loc_bf, vn_ps)

                # AllGather K (padded) + V (natural), bf16
                kv_in = dram.tile([416, 128], BF16, name=f"kv_in_{l}")
                for t3 in range(3):
                    nc.sync.dma_start(kv_in[96 * t3:96 * (t3 + 1), :], kp_loc[t3])
                nc.sync.dma_start(kv_in[288:416, :], v_loc_bf)
                kv_out = dram.tile([NCORES * 416, 128], BF16,
                                   addr_space="Shared", name=f"kv_out_{l}")
                nc.gpsimd.collective_compute(
                    "AllGather", mybir.AluOpType.bypass,
                    replica_groups=[list(range(NCORES))],
                    ins=[kv_in.opt()], outs=[kv_out.opt()],
                )
                kp_full = []
                kvr = kv_out.rearrange("(c r) f -> r c f", r=416)
                for t3 in range(3):
                    kf = sb.tile([96, N], BF16, name=f"kpfull_{l}_{t3}", bufs=1)
                    nc.sync.dma_start(
                        kf.rearrange("p (c f) -> p c f", c=NCORES),
                        kvr[96 * t3:96 * (t3 + 1), :, :])
                    kp_full.append(kf)
                v_bf = sb.tile([128, N], BF16, name=f"vbf_{l}", bufs=1)
                nc.sync.dma_start(v_bf.rearrange("p (c f) -> p c f", c=NCORES),
                                  kvr[288:416, :, :])

                # attention
                o_ps = ps_o.tile([NL, E], F32, name=f"o_ps_{l}")
                rec_all = sb.tile([NL, NH], F32, name=f"rec_{l}")
                for h in range(NH):
                    hs = slice(HD * h, HD * (h + 1))
                    t3, r3 = h // 3, h % 3
                    prow = slice(32 * r3, 32 * r3 + 32)
                    exp_sb = sbT.tile([NL, N], BF16, name=f"exp_{l}_{h}", tag="exp",
                                      bufs=2)
                    sum_h = sbT.tile([NL, 1], F32, name=f"sumh_{l}_{h}", tag="sumh",
                                     bufs=2)
                    s_ps = ps_s.tile([NL, N], F32, name=f"s_ps_{l}_{h}", tag="s_ps")
                    for j in range(2):
                        nc.tensor.matmul(s_ps[:, 512 * j:512 * (j + 1)],
                                         qp[t3][prow, :],
                                         kp_full[t3][prow, 512 * j:512 * (j + 1)],
                                         start=True, stop=True)
                    nc.scalar.activation(exp_sb, s_ps, Act.Exp, accum_out=sum_h)
                    nc.vector.reciprocal(rec_all[:, h:h + 1], sum_h)
                    for mc in range(NCORES):
                        tp_ps = ps_t.tile([128, 128], BF16, name=f"tp_{l}_{h}_{mc}",
                                          tag="tp")
                        nc.tensor.transpose(
                            tp_ps, exp_sb[:, 128 * mc:128 * (mc + 1)], ident_b)
                        expT = sbT.tile([128, 128], BF16, name=f"expT_{l}_{h}_{mc}",
                                        tag="expT", bufs=3)
                        nc.vector.tensor_copy(expT, tp_ps)
                        nc.tensor.matmul(
                            o_ps[:, hs],
                            expT,
                            v_bf[:, 128 * mc + HD * h:128 * mc + HD * (h + 1)],
                            start=(mc == 0), stop=(mc == NCORES - 1))

                o_sb = sb.tile([NL, E], F32, name=f"o_sb_{l}")
                for h in range(NH):
                    hs = slice(HD * h, HD * (h + 1))
                    nc.vector.tensor_scalar(o_sb[:, hs], o_ps[:, hs],
                                            rec_all[:, h:h + 1], None, op0=Alu.mult)

                # out projection (natural layout): lhsT = oT, rhs = out_wT
                oT_ps = ps_m.tile([E, NL], F32, name=f"oT_ps_{l}", tag="m")
                nc.tensor.transpose(oT_ps, o_sb, ident_f)
                oT = sb.tile([E, NL], F32, name=f"oT_{l}")
                nc.vector.tensor_copy(oT, oT_ps)
                op_ps = ps_m.tile([NL, E], F32, name=f"op_ps_{l}", tag="m")
                nc.tensor.matmul(op_ps, oT, W("owT", l), start=True, stop=True)

                # residual + layernorm (natural layout)
                x_sb = sb.tile([NL, E], F32, name=f"x_{l}")
                nc.vector.tensor_tensor(x_sb, feat_nat, op_ps, Alu.add)
                nc.vector.tensor_tensor(x_sb, x_sb, BC(f"out_b_{l}"), Alu.add)
                stats = sb.tile([NL, 6], F32, name=f"stats_{l}")
                nc.vector.bn_stats(stats, x_sb)
                mv = sb.tile([NL, 2], F32, name=f"mv_{l}")
                nc.vector.bn_aggr(mv, stats)
                std = sb.tile([NL, 1], F32, name=f"std_{l}")
                nc.scalar.activation(std, mv[:, 1:2], Act.Sqrt, bias=eps_col)
                rstd = sb.tile([NL, 1], F32, name=f"rstd_{l}")
                nc.vector.reciprocal(rstd, std)
                nc.vector.tensor_scalar(x_sb, x_sb, mv[:, 0:1], rstd,
                                        op0=Alu.subtract, op1=Alu.mult)
                nc.vector.tensor_tensor(x_sb, x_sb, BC(f"ln_w_{l}"), Alu.mult)
                nc.vector.tensor_tensor(feat_nat, x_sb, BC(f"ln_b_{l}"), Alu.add)

                fT_ps = ps_m.tile([E, NL], F32, name=f"fT_ps_{l}", tag="m")
                nc.tensor.transpose(fT_ps, feat_nat, ident_f)
                nc.vector.tensor_copy(featT, fT_ps)

                # MLP (gelu via tanh approximation, sim-supported)
                o2_ps = ps_o.tile([NL, E], F32, name=f"o2_ps_{l}")
                for j in range(2):
                    m_ps = ps_m.tile([128, NL], F32, name=f"m_ps_{l}_{j}", tag="m")
                    nc.tensor.matmul(m_ps, W("w1T", l)[:, 128 * j:128 * (j + 1)],
                                     featT, start=True, stop=True)
                    xg = sb.tile([128, NL], F32, name=f"xg_{l}_{j}", tag="xg",
                                 bufs=2)
                    nc.scalar.activation(xg, m_ps, Act.Identity,
                                         bias=W("b1", l, j))
                    x2 = sb.tile([128, NL], F32, name=f"x2_{l}_{j}", tag="x2",
                                 bufs=2)
                    nc.vector.tensor_tensor(x2, xg, xg, Alu.mult)
                    nc.vector.tensor_tensor(x2, x2, xg, Alu.mult)
                    nc.vector.tensor_scalar(x2, x2, 0.044715, None, op0=Alu.mult)
                    nc.vector.tensor_tensor(x2, x2, xg, Alu.add)
                    tg = sb.tile([128, NL], F32, name=f"tg_{l}_{j}", tag="tg",
                                 bufs=2)
                    nc.scalar.activation(tg, x2, Act.Tanh,
                                         scale=0.7978845608028654)
                    mT = sb.tile([128, NL], F32, name=f"mT_{l}_{j}", bufs=3)
                    nc.vector.tensor_tensor(mT, xg, tg, Alu.mult)
                    nc.vector.tensor_tensor(mT, mT, xg, Alu.add)
                    nc.vector.tensor_scalar(mT, mT, 0.5, None, op0=Alu.mult)
                    nc.tensor.matmul(o2_ps, mT, W("w2T", l, j),
                                     start=(j == 0), stop=(j == 1))
                nc.vector.tensor_tensor(feat_nat, feat_nat, o2_ps, Alu.add)
                nc.vector.tensor_tensor(feat_nat, feat_nat, BC(f"mlp_b2_{l}"),
                                        Alu.add)

        # final featT
        with tc.tile_pool(name="ps_fin", bufs=2, space="PSUM") as ps_fin:
            fT_ps = ps_fin.tile([E, NL], F32, name="fT_ps", tag="fin")
            nc.tensor.transpose(fT_ps, feat_nat, ident_f)
            nc.vector.tensor_copy(featT, fT_ps)

            # ---------------- pair head prep (before seg/bot heads) ----------
            # T-layouts (signed-scaled): a2T = w2*aT ; b2sT = w2*(bT + b1)
            aT_ps = ps_fin.tile([HEAD, NL], F32, name="aT_ps", tag="fin")
            nc.tensor.matmul(aT_ps, W("WaT"), featT, start=True, stop=True)
            a2T_bf = persist.tile([HEAD, NL], BF16)
            nc.vector.tensor_scalar(a2T_bf, aT_ps, W("w2c"), None, op0=Alu.mult)
            bT_ps = ps_fin.tile([HEAD, NL], F32, name="bT_ps", tag="fin")
            nc.tensor.matmul(bT_ps, W("WbT"), featT, start=True, stop=True)
            b2sT_bf = persist.tile([HEAD, NL], BF16)
            nc.vector.tensor_scalar(b2sT_bf, bT_ps, W("b1c"), W("w2c"),
                                    op0=Alu.add, op1=Alu.mult)

            # natural layouts, scaled+stacked, then AG
            an_ps = ps_fin.tile([NL, HEAD], F32, name="an_ps", tag="fin")
            nc.tensor.matmul(an_ps, featT, W("WaT"), start=True, stop=True)
            bn_ps = ps_fin.tile([NL, HEAD], F32, name="bn_ps", tag="fin")
            nc.tensor.matmul(bn_ps, featT, W("WbT"), start=True, stop=True)
            ab_nat = persist.tile([NL, 2 * HEAD], F32)
            nc.vector.tensor_copy(ab_nat[:, 0:HEAD], an_ps)
            nc.vector.tensor_copy(ab_nat[:, HEAD:2 * HEAD], bn_ps)
            nc.vector.tensor_tensor(ab_nat, ab_nat, BC("baddrow"), Alu.add)
            nc.vector.tensor_tensor(ab_nat, ab_nat, BC("w2row"), Alu.mult)
            ab_bf = persist.tile([NL, 2 * HEAD], BF16)
            nc.vector.tensor_copy(ab_bf, ab_nat)

            ab_in = dram.tile([NL, 2 * HEAD], BF16)
            nc.sync.dma_start(ab_in, ab_bf)
            ab_out = dram.tile([N, 2 * HEAD], BF16, addr_space="Shared")
            nc.gpsimd.collective_compute(
                "AllGather", mybir.AluOpType.bypass,
                replica_groups=[list(range(NCORES))],
                ins=[ab_in.opt()], outs=[ab_out.opt()],
            )

            # slab source in DRAM: rows 0-63 a2T, 64-127 b2sT (bf16)
            slab_src = dram.tile([2 * HEAD, NL], BF16)
            nc.sync.dma_start(slab_src[0:HEAD, :], a2T_bf)
            nc.sync.dma_start(slab_src[HEAD:2 * HEAD, :], b2sT_bf)

            # combined slabs: a-set at cols [0, 8192), b-set at [8192, 16384)
            slabs_ab = persist.tile([128, 2 * HEAD * NL], BF16)
            for g in range(2 * HEAD // 8):
                nc.sync.dma_start(
                    slabs_ab[:, NL * 8 * g:NL * 8 * (g + 1)].rearrange(
                        "p (h f) -> p h f", h=8),
                    part_bcast(slab_src[8 * g:8 * (g + 1), :]))

            cols_bf = persist.tile([128, NCORES * 2 * HEAD], BF16)
            nc.sync.dma_start(cols_bf.rearrange("p (c f) -> p c f", c=NCORES),
                              ab_out.rearrange("(c r) f -> r c f", r=128))
            cols_f = persist.tile([128, NCORES * 2 * HEAD], F32)
            nc.vector.tensor_copy(cols_f, cols_bf)

            # ---------------- seg / bottom heads ----------------
            with tc.tile_pool(name="head_sb", bufs=2) as hsb:
                s1_ps = ps_fin.tile([HEAD, NL], F32, name="s1_ps", tag="fin")
                nc.tensor.matmul(s1_ps, W("sw1T"), featT, start=True, stop=True)
                s1T = hsb.tile([HEAD, NL], F32)
                nc.scalar.activation(s1T, s1_ps, Act.Relu, bias=W("sb1"))
                s2_ps = ps_fin.tile([NCLS, NL], F32, name="s2_ps", tag="fin")
                nc.tensor.matmul(s2_ps, W("sw2T"), s1T, start=True, stop=True)
                segT = hsb.tile([NCLS, NL], F32)
                nc.scalar.activation(segT, s2_ps, Act.Identity, bias=W("sb2"))
                nc.sync.dma_start(segT_out[:, :], segT)

                t1_ps = ps_fin.tile([HEAD, NL], F32, name="t1_ps", tag="fin")
                nc.tensor.matmul(t1_ps, W("bw1T"), featT, start=True, stop=True)
                t1T = hsb.tile([HEAD, NL], F32)
                nc.scalar.activation(t1T, t1_ps, Act.Relu, bias=W("bb1"))
                t2_ps = ps_fin.tile([1, NL], F32, name="t2_ps", tag="fin")
                nc.tensor.matmul(t2_ps, W("bw2T"), t1T, start=True, stop=True)
                botT = hsb.tile([1, NL], F32)
                nc.scalar.activation(botT, t2_ps, Act.Identity, bias=W("bb2"))
                nc.sync.dma_start(bot_out[:, :], botT)

        # ------- pair head main loop: R and M passes per chunk, wide ops -------
        # h-axis sorted (positive w2 first). Groups of 8 h-tiles:
        #   ACT groups: 8x [128,128] Relu ops (positive h only)
        #   DVE groups: wide TT-add (cols broadcast via 0-stride) + wide TS max/min
        # all accumulated by PE identity-matmuls into a [128, 512] PSUM (4 lanes),
        # folded at the end. Chunk result = mR*R + mM*M + mB.
        n_groups = HEAD // 8
        n_act_groups = min(N_ACT_GROUPS, npos // 8)
        with tc.tile_pool(name="pair_ps", bufs=3, space="PSUM") as pps, \
             tc.tile_pool(name="pair_sb", bufs=8) as psb, \
             tc.tile_pool(name="mask_sb", bufs=2) as msb, \
             tc.tile_pool(name="out_sb", bufs=2) as osb:

            def chunk_pass(jc, pas):
                soff = 0 if pas == "R" else HEAD * NL          # slab set
                coff = 128 * jc + (HEAD if pas == "R" else 0)  # col half
                acc_ps = pps.tile([128, 512], F32, name=f"accps_{jc}_{pas}",
                                  tag="accps")
                for g in range(n_groups):
                    wb = psb.tile([128, 8 * NL], BF16, name=f"wb_{jc}_{pas}_{g}",
                                  tag="wb", bufs=4)
                    if g < n_act_groups:
                        for u in range(8):
                            h = 8 * g + u
                            nc.scalar.activation(
                                wb[:, NL * u:NL * (u + 1)],
                                slabs_ab[:, soff + NL * h:soff + NL * (h + 1)],
                                Act.Relu,
                                bias=cols_f[:, coff + h:coff + h + 1])
                    else:
                        h0 = 8 * g
                        csl = cols_bf[:, coff + h0:coff + h0 + 8]
                        cb = bass.AP(tensor=csl.tensor, offset=csl.offset,
                                     ap=[list(csl.ap[0]), list(csl.ap[1]),
                                         [0, NL]])
                        eng = nc.gpsimd if g == n_groups - 1 else nc.vector
                        eng.tensor_tensor(
                            wb.rearrange("p (h f) -> p h f", h=8),
                            slabs_ab[:, soff + NL * h0:soff + NL * (h0 + 8)]
                            .rearrange("p (h f) -> p h f", h=8),
                            cb, Alu.add)
                        # relu: max for positive-w2 h's, min for negative
                        lo, hi = h0, h0 + 8
                        if hi <= npos or lo >= npos:
                            op1 = Alu.max if lo < npos else Alu.min
                            eng.tensor_scalar(wb, wb, 0.0, None, op0=op1)
                        else:
                            kpos = npos - lo
                            nc.vector.tensor_scalar(
                                wb[:, :NL * kpos], wb[:, :NL * kpos], 0.0, None,
                                op0=Alu.max)
                            nc.vector.tensor_scalar(
                                wb[:, NL * kpos:], wb[:, NL * kpos:], 0.0, None,
                                op0=Alu.min)
                    for q in range(2):
                        nc.tensor.matmul(acc_ps, ident_b,
                                         wb[:, 512 * q:512 * (q + 1)],
                                         start=(g == 0 and q == 0),
                                         stop=(g == n_groups - 1 and q == 1))
                cp = psb.tile([128, 512], F32, name=f"cp_{jc}_{pas}", tag="cp",
                              bufs=3)
                nc.scalar.copy(cp, acc_ps)
                f1 = psb.tile([128, 256], F32, name=f"f1_{jc}_{pas}", tag="f1",
                              bufs=3)
                nc.vector.tensor_tensor(f1, cp[:, 0:256], cp[:, 256:512], Alu.add)
                tot = psb.tile([128, NL], F32, name=f"tot_{jc}_{pas}", tag="tot",
                               bufs=3)
                nc.vector.tensor_tensor(tot, f1[:, 0:128], f1[:, 128:256], Alu.add)
                return tot

            for jc in range(NCORES):
                totR = chunk_pass(jc, "R")
                totM = chunk_pass(jc, "M")
                m3 = msb.tile([128, 3 * NL], F32, name=f"m3_{jc}", tag="m3")
                nc.sync.dma_start(m3, P["masks3"][128 * jc:128 * (jc + 1), :])
                t1 = osb.tile([128, NL], F32, name=f"t1_{jc}", tag="t1")
                nc.vector.tensor_tensor(t1, totR, m3[:, 0:NL], Alu.mult)
                t2 = osb.tile([128, NL], F32, name=f"t2_{jc}", tag="t2")
                nc.vector.tensor_tensor(t2, totM, m3[:, NL:2 * NL], Alu.mult)
                nc.vector.tensor_tensor(t1, t1, t2, Alu.add)
                nc.vector.tensor_tensor(t1, t1, m3[:, 2 * NL:3 * NL], Alu.add)
                nc.sync.dma_start(instT_out[128 * jc:128 * (jc + 1), :], t1)

    nc.compile()
    return nc


def _blob_spec():
    """Column layout of the packed parameter blob: key -> (partitions, c0, c1)."""
    spec = {}
    col = 0

    def add(key, p, f):
        nonlocal col
        spec[key] = (p, col, col + f)
        col += f

    add("xT", NODE_ATTR + GRID, NL)
    add("emb_wT", NODE_ATTR + GRID, E)
    add("emb_b", E, 1)
    for l in range(L):
        add(f"ipT_{l}", E, 704)
        for j in range(6):
            add(f"ipb_{l}_{j}", 96, 1)
        add(f"ipb_{l}_6", 128, 1)
        add(f"owT_{l}", E, E)
        add(f"w1T_{l}", E, MLP_H)
        for j in range(2):
            add(f"b1_{l}_{j}", 128, 1)
        for j in range(2):
            add(f"w2T_{l}_{j}", 128, E)
    add("sw1T", E, HEAD)
    add("sb1", HEAD, 1)
    add("sw2T", HEAD, NCLS)
    add("sb2", NCLS, 1)
    add("bw1T", E, HEAD)
    add("bb1", HEAD, 1)
    add("bw2T", HEAD, 1)
    add("bb2", 1, 1)
    add("WaT", E, HEAD)
    add("WbT", E, HEAD)
    add("w2c", HEAD, 1)
    add("b1c", HEAD, 1)
    return col, spec


def _host_prep(inputs):
    f = np.float32
    x = np.asarray(inputs["x"], f)
    node_w = np.asarray(inputs["node_w"], f)
    node_b = np.asarray(inputs["node_b"], f)
    grid_w = np.asarray(inputs["grid_w"], f)
    grid_b = np.asarray(inputs["grid_b"], f)

    emb_wT = np.zeros((NODE_ATTR + GRID, E), f)
    emb_wT[:NODE_ATTR, :64] = node_w.T
    emb_wT[NODE_ATTR:, 64:] = grid_w.T
    emb_b = np.concatenate([node_b, grid_b])[:, None].astype(f)

    # head-padded q/k: head h -> rows 96*(h//3) + 32*(h%3) .. +16
    ipw = np.asarray(inputs["in_proj_w"], f)
    ipb = np.asarray(inputs["in_proj_b"], f)
    qs = f(1.0 / np.sqrt(HD))
    qpad = np.zeros((L, 288, E), f)
    kpad = np.zeros((L, 288, E), f)
    bq = np.zeros((L, 288), f)
    bk = np.zeros((L, 288), f)
    for h in range(NH):
        dst = slice(96 * (h // 3) + 32 * (h % 3), 96 * (h // 3) + 32 * (h % 3) + HD)
        qpad[:, dst, :] = ipw[:, HD * h:HD * (h + 1), :] * qs
        bq[:, dst] = ipb[:, HD * h:HD * (h + 1)] * qs
        kpad[:, dst, :] = ipw[:, E + HD * h:E + HD * (h + 1), :]
        bk[:, dst] = ipb[:, E + HD * h:E + HD * (h + 1)]
    ipT = np.concatenate([qpad.transpose(0, 2, 1), kpad.transpose(0, 2, 1),
                          ipw[:, 2 * E:, :].transpose(0, 2, 1)], axis=2)
    ipb_pad = np.concatenate([bq, bk, ipb[:, 2 * E:]], axis=1)  # [L, 704]

    # pair head: sort h-axis by sign of w2 (positive first)
    inst_w1 = np.asarray(inputs["inst_w1"], f)
    w2 = np.asarray(inputs["inst_w2"], f)[0]
    b1i = np.asarray(inputs["inst_b1"], f)
    b2i = float(np.asarray(inputs["inst_b2"], f)[0])
    perm = np.argsort((w2 <= 0).astype(np.int32), kind="stable")
    npos = int((w2 > 0).sum())
    w2_s = w2[perm]
    b1_s = b1i[perm]
    WaT = np.ascontiguousarray(inst_w1[perm][:, :E].T)
    WbT = np.ascontiguousarray(inst_w1[perm][:, E:].T)

    blob_cols, spec = _blob_spec()
    blob = np.zeros((128, blob_cols), f)

    def put(key, arr):
        p, c0, c1 = spec[key]
        blob[:p, c0:c1] = np.asarray(arr, f).reshape(p, c1 - c0)

    put("emb_wT", emb_wT)
    put("emb_b", emb_b)
    for l in range(L):
        put(f"ipT_{l}", ipT[l])
        for j in range(6):
            put(f"ipb_{l}_{j}", ipb_pad[l, 96 * j:96 * (j + 1), None])
        put(f"ipb_{l}_6", ipb_pad[l, 576:704, None])
        put(f"owT_{l}", np.asarray(inputs["out_w"], f)[l].T)
        put(f"w1T_{l}", np.asarray(inputs["mlp_w1"], f)[l].T)
        for j in range(2):
            put(f"b1_{l}_{j}",
                np.asarray(inputs["mlp_b1"], f)[l, 128 * j:128 * (j + 1), None])
            put(f"w2T_{l}_{j}",
                np.asarray(inputs["mlp_w2"], f)[l].T[128 * j:128 * (j + 1), :])
    put("sw1T", np.asarray(inputs["seg_w1"], f).T)
    put("sb1", np.asarray(inputs["seg_b1"], f)[:, None])
    put("sw2T", np.asarray(inputs["seg_w2"], f).T)
    put("sb2", np.asarray(inputs["seg_b2"], f)[:, None])
    put("bw1T", np.asarray(inputs["bot_w1"], f).T)
    put("bb1", np.asarray(inputs["bot_b1"], f)[:, None])
    put("bw2T", np.asarray(inputs["bot_w2"], f).T)
    put("bb2", np.asarray(inputs["bot_b2"], f).reshape(1, 1))
    put("WaT", WaT)
    put("WbT", WbT)
    put("w2c", w2_s[:, None])
    put("b1c", b1_s[:, None])

    rows14 = np.zeros((14, E), f)
    rows14[0:3] = np.asarray(inputs["out_b"], f)
    rows14[3:6] = np.asarray(inputs["ln_w"], f)
    rows14[6:9] = np.asarray(inputs["ln_b"], f)
    rows14[9:12] = np.asarray(inputs["mlp_b2"], f)
    rows14[12] = np.concatenate([w2_s, w2_s])
    rows14[13] = np.concatenate([np.zeros(HEAD, f), b1_s])

    gj = np.arange(N)[:, None]
    in_maps = []
    for c in range(NCORES):
        b = blob.copy()
        p, c0, c1 = spec["xT"]
        b[:p, c0:c1] = x[NL * c:NL * (c + 1), :].T

        # masks: mR selects M[gi, j] (j > gi), mM selects M[j, gi] (j < gi)
        gi = (NL * c + np.arange(NL))[None, :]
        mR = (gj > gi).astype(f)
        mM = (gj < gi).astype(f)
        m3 = np.concatenate([mR, mM, b2i * (mR + mM)], axis=1)

        in_maps.append({"blob": b, "rows14": rows14, "masks3": m3})

    return in_maps, npos


def _get_graph(npos):
    if npos not in _GRAPH_CACHE:
        _GRAPH_CACHE[npos] = _build(npos)
    return _GRAPH_CACHE[npos]


def run_on_hw(inputs, trace=False):
    import sys
    if "/opt/trn_rl_repo" not in sys.path:
        sys.path.insert(0, "/opt/trn_rl_repo")
    from concourse.bass_utils import run_bass_kernel_spmd
    in_maps, npos = _host_prep(inputs)
    nc = _get_graph(npos)
    return run_bass_kernel_spmd(nc, in_maps, core_ids=list(range(NCORES)),
                                trace=trace)


def _assemble(results):
    f = np.float32
    seg = np.concatenate([np.asarray(r["segT_out"], f).T for r in results], axis=0)
    bottom = np.concatenate([np.asarray(r["bot_out"], f)[0] for r in results])
    inst = np.empty((N, N), f)
    for c, r in enumerate(results):
        inst[:, NL * c:NL * (c + 1)] = np.asarray(r["instT_out"], f)
    return seg, inst, bottom


def kernel(**inputs):
    res = run_on_hw(inputs, trace=False)
    return _assemble(res.results)


# revision 20
# speedup vs baseline: 1.5715x; 1.5715x over previous
"""Trainium2 Bass kernel for AAGNetSegmentor (1024-face graph transformer + all-pairs
instance head), SPMD across 8 NeuronCores.

Sharding: faces (N=1024) split into 8 row-blocks of 128. Backbone is sequence-parallel
with per-layer AllGather of K/V (bf16, head-padded to 32-partition alignment). The
N x N pair head is computed as per-core COLUMN blocks of the symmetric output (each
core computes [all j, own i] via row symmetry). Each j-chunk needs only ONE variant
(upper or lower), selected per-core at runtime through register-offset APs driven by
a host-provided table; the diagonal chunk is completed with its own transpose plus
triangular masks.

kernel(**inputs) takes the full unsharded inputs and returns
(seg_out [1024,25], inst_matrix [1024,1024], bottom_out [1024]) as float32.
"""

import numpy as np

# problem constants
N = 1024
NODE_ATTR = 10
GRID = 7
E = 128
NH = 8
HD = 16
L = 3
MLP_H = 256
HEAD = 64
NCLS = 25
LN_EPS = 1e-5

NCORES = 8
NL = N // NCORES  # 128

# pair-head engine assignment (h-axis is host-sorted: positive w2 first)
N_ACT_GROUPS = 2  # groups of 8 h's relu'd on ACT (needs 8*N_ACT_GROUPS <= npos)

_GRAPH_CACHE = {}


def _build(npos):
    """npos: number of positive-w2 h's (h-axis pre-sorted: 0..npos-1 positive)."""
    import concourse.bass as bass
    import concourse.bacc as bacc
    import concourse.tile as tile
    import concourse.mybir as mybir
    from concourse.masks import make_identity
    from contextlib import ExitStack

    F32 = mybir.dt.float32
    BF16 = mybir.dt.bfloat16
    I32 = mybir.dt.int32
    Alu = mybir.AluOpType
    Act = mybir.ActivationFunctionType

    nc = bacc.Bacc("TRN2", target_bir_lowering=False)

    # ---------------- external I/O ----------------
    blob_cols, blob_spec = _blob_spec()
    P = {}

    def par(name, shape, dtype=F32):
        P[name] = nc.declare_dram_parameter(name, list(shape), dtype, isOutput=False)
        return P[name]

    par("blob", [128, blob_cols])
    par("rows14", [14, E])          # broadcast rows (ln/out_b/mlp_b2/w2/badd)
    par("masks3", [N, 3 * NL])      # per chunk: [mR | mM | mB]

    segT_out = nc.declare_dram_parameter("segT_out", [NCLS, NL], F32, isOutput=True)
    bot_out = nc.declare_dram_parameter("bot_out", [1, NL], F32, isOutput=True)
    instT_out = nc.declare_dram_parameter("instT_out", [N, NL], F32, isOutput=True)

    with tile.TileContext(nc) as tc, ExitStack() as top:
        wpool = top.enter_context(tc.tile_pool(name="weights", bufs=1))
        cpool = top.enter_context(tc.tile_pool(name="consts", bufs=1))
        persist = top.enter_context(tc.tile_pool(name="persist", bufs=1))
        dram = top.enter_context(tc.tile_pool(name="dram", bufs=1, space="DRAM"))

        # ---------------- params: one blob DMA; weights = slices ----------------
        blob = wpool.tile([128, blob_cols], F32)
        nc.sync.dma_start(blob, P["blob"][:, :])

        def W(name, l=None, j=None):
            key = name if l is None else (f"{name}_{l}" if j is None
                                          else f"{name}_{l}_{j}")
            p, c0, c1 = blob_spec[key]
            return blob[0:p, c0:c1]

        # broadcast tiles from rows14: one batched broadcast DMA
        rows_bc = cpool.tile([128, 14 * E], F32)

        def part_bcast(ap_slice, parts=128):
            return bass.AP(tensor=ap_slice.tensor, offset=ap_slice.offset,
                           ap=[[0, parts]] + [list(p) for p in ap_slice.ap])

        nc.sync.dma_start(rows_bc.rearrange("p (r f) -> p r f", r=14),
                          part_bcast(P["rows14"][:, :]))
        ROWS = ["out_b_0", "out_b_1", "out_b_2", "ln_w_0", "ln_w_1", "ln_w_2",
                "ln_b_0", "ln_b_1", "ln_b_2", "mlp_b2_0", "mlp_b2_1", "mlp_b2_2",
                "w2row", "baddrow"]

        def BC(key):
            r = ROWS.index(key)
            return rows_bc[:, E * r:E * (r + 1)]

        ident_f = cpool.tile([128, 128], F32)
        make_identity(nc, ident_f)
        ident_b = cpool.tile([128, 128], BF16)
        make_identity(nc, ident_b)
        eps_col = cpool.tile([NL, 1], F32)
        nc.vector.memset(eps_col, LN_EPS)

        # ---------------- embed ----------------
        feat_nat = persist.tile([NL, E], F32)
        featT = persist.tile([E, NL], F32)

        with tc.tile_pool(name="emb_ps", bufs=2, space="PSUM") as eps_pool:
            ft_ps = eps_pool.tile([E, NL], F32)
            nc.tensor.matmul(ft_ps, W("emb_wT"), W("xT"), start=True, stop=True)
            nc.scalar.activation(featT, ft_ps, Act.Identity, bias=W("emb_b"))
            fn_ps = eps_pool.tile([NL, E], F32)
            nc.tensor.transpose(fn_ps, featT, ident_f)
            nc.vector.tensor_copy(feat_nat, fn_ps)

        # ---------------- transformer layers ----------------
        for l in range(L):
            with ExitStack() as lyr:
                ps_m = lyr.enter_context(
                    tc.tile_pool(name=f"ps_m_{l}", bufs=2, space="PSUM"))
                ps_s = lyr.enter_context(
                    tc.tile_pool(name=f"ps_s_{l}", bufs=1, space="PSUM"))
                ps_o = lyr.enter_context(
                    tc.tile_pool(name=f"ps_o_{l}", bufs=1, space="PSUM"))
                sb = lyr.enter_context(tc.tile_pool(name=f"sb_{l}", bufs=2))
                sbT = lyr.enter_context(tc.tile_pool(name=f"sbT_{l}", bufs=3))

                # qkv; q/k head-padded: 3 tiles of [96, NL] each
                qp = []
                kp_loc = []
                for t3 in range(3):
                    q_ps = ps_m.tile([96, NL], F32, name=f"q_ps_{l}_{t3}", tag="m")
                    nc.tensor.matmul(q_ps, W("ipT", l)[:, 96 * t3:96 * (t3 + 1)],
                                     featT, start=True, stop=True)
                    qt = sb.tile([96, NL], BF16, name=f"qp_{l}_{t3}", bufs=2)
                    nc.scalar.activation(qt, q_ps, Act.Identity,
                                         bias=W("ipb", l, t3))
                    qp.append(qt)
                    k_ps = ps_m.tile([96, NL], F32, name=f"k_ps_{l}_{t3}", tag="m")
                    nc.tensor.matmul(
                        k_ps, W("ipT", l)[:, 288 + 96 * t3:288 + 96 * (t3 + 1)],
                        featT, start=True, stop=True)
                    kt = sb.tile([96, NL], BF16, name=f"kp_{l}_{t3}", bufs=2)
                    nc.scalar.activation(kt, k_ps, Act.Identity,
                                         bias=W("ipb", l, 3 + t3))
                    kp_loc.append(kt)
                v_ps = ps_m.tile([E, NL], F32, name=f"v_ps_{l}", tag="m")
                nc.tensor.matmul(v_ps, W("ipT", l)[:, 576:704], featT,
                                 start=True, stop=True)
                vT = sb.tile([E, NL], F32, name=f"vT_{l}", bufs=2)
                nc.scalar.activation(vT, v_ps, Act.Identity, bias=W("ipb", l, 6))
                vn_ps = ps_m.tile([NL, E], F32, name=f"vn_ps_{l}", tag="m")
                nc.tensor.transpose(vn_ps, vT, ident_f)
                v_loc_bf = sb.tile([NL, E], BF16, name=f"vlocbf_{l}")
                nc.vector.tensor_copy(v_loc_bf, vn_ps)

                # AllGather K (padded) + V (natural), bf16
                kv_in = dram.tile([416, 128], BF16, name=f"kv_in_{l}")
                for t3 in range(3):
                    nc.sync.dma_start(kv_in[96 * t3:96 * (t3 + 1), :], kp_loc[t3])
                nc.sync.dma_start(kv_in[288:416, :], v_loc_bf)
                kv_out = dram.tile([NCORES * 416, 128], BF16,
                                   addr_space="Shared", name=f"kv_out_{l}")
                nc.gpsimd.collective_compute(
                    "AllGather", mybir.AluOpType.bypass,
                    replica_groups=[list(range(NCORES))],
                    ins=[kv_in.opt()], outs=[kv_out.opt()],
                )
                kp_full = []
                kvr = kv_out.rearrange("(c r) f -> r c f", r=416)
                for t3 in range(3):
                    kf = sb.tile([96, N], BF16, name=f"kpfull_{l}_{t3}", bufs=1)
                    nc.sync.dma_start(
                        kf.rearrange("p (c f) -> p c f", c=NCORES),
                        kvr[96 * t3:96 * (t3 + 1), :, :])
                    kp_full.append(kf)
                # v_aug: per m-chunk 136 cols = 8 heads x (16 v-cols + ones col)
                v_aug = sb.tile([128, NCORES * 136], BF16, name=f"vaug_{l}", bufs=1)
                nc.vector.memset(v_aug, 1.0)
                for c in range(NCORES):
                    va = bass.AP(tensor=v_aug.tensor,
                                 offset=v_aug.offset + 136 * c,
                                 ap=[list(v_aug.ap[0]), [17, NH], [1, HD]])
                    nc.sync.dma_start(
                        va,
                        kvr[288:416, c, :].rearrange("p (h d) -> p h d", h=NH))

                # attention: scores computed TRANSPOSED (m on partitions), so
                # no per-chunk transposes are needed before AV. Four [128,128]
                # sT tiles pack into one [128,512] PSUM so exp stays wide. The
                # ones column in v_aug yields softmax sums during AV.
                o_ps = ps_o.tile([NL, NH * 17], F32, name=f"o_ps_{l}")
                rec_all = sb.tile([NL, NH], F32, name=f"rec_{l}")
                for h in range(NH):
                    t3, r3 = h // 3, h % 3
                    prow = slice(32 * r3, 32 * r3 + 32)
                    expT_sb = sbT.tile([128, N], BF16, name=f"expT_{l}_{h}",
                                       tag="exp", bufs=2)
                    for half in range(2):
                        sT_ps = ps_s.tile([NL, 512], F32,
                                          name=f"sT_ps_{l}_{h}_{half}", tag="s_ps",
                                          bufs=2)
                        for u in range(4):
                            mc = 4 * half + u
                            nc.tensor.matmul(
                                sT_ps[:, 128 * u:128 * (u + 1)],
                                kp_full[t3][prow, 128 * mc:128 * (mc + 1)],
                                qp[t3][prow, :], start=True, stop=True)
                        nc.scalar.activation(
                            expT_sb[:, 512 * half:512 * (half + 1)], sT_ps,
                            Act.Exp)
                    for mc in range(NCORES):
                        nc.tensor.matmul(
                            o_ps[:, 17 * h:17 * (h + 1)],
                            expT_sb[:, 128 * mc:128 * (mc + 1)],
                            v_aug[:, 136 * mc + 17 * h:136 * mc + 17 * (h + 1)],
                            start=(mc == 0), stop=(mc == NCORES - 1))
                    nc.vector.reciprocal(rec_all[:, h:h + 1],
                                         o_ps[:, 17 * h + 16:17 * h + 17])

                o_sb = sb.tile([NL, E], F32, name=f"o_sb_{l}")
                for h in range(NH):
                    nc.vector.tensor_scalar(o_sb[:, HD * h:HD * (h + 1)],
                                            o_ps[:, 17 * h:17 * h + 16],
                                            rec_all[:, h:h + 1], None, op0=Alu.mult)

                # out projection (natural layout): lhsT = oT, rhs = out_wT
                oT_ps = ps_m.tile([E, NL], F32, name=f"oT_ps_{l}", tag="m")
                nc.tensor.transpose(oT_ps, o_sb, ident_f)
                oT = sb.tile([E, NL], F32, name=f"oT_{l}")
                nc.vector.tensor_copy(oT, oT_ps)
                op_ps = ps_m.tile([NL, E], F32, name=f"op_ps_{l}", tag="m")
                nc.tensor.matmul(op_ps, oT, W("owT", l), start=True, stop=True)

                # residual + layernorm (natural layout)
                x_sb = sb.tile([NL, E], F32, name=f"x_{l}")
                nc.vector.tensor_tensor(x_sb, feat_nat, op_ps, Alu.add)
                nc.vector.tensor_tensor(x_sb, x_sb, BC(f"out_b_{l}"), Alu.add)
                stats = sb.tile([NL, 6], F32, name=f"stats_{l}")
                nc.vector.bn_stats(stats, x_sb)
                mv = sb.tile([NL, 2], F32, name=f"mv_{l}")
                nc.vector.bn_aggr(mv, stats)
                std = sb.tile([NL, 1], F32, name=f"std_{l}")
                nc.scalar.activation(std, mv[:, 1:2], Act.Sqrt, bias=eps_col)
                rstd = sb.tile([NL, 1], F32, name=f"rstd_{l}")
                nc.vector.reciprocal(rstd, std)
                nc.vector.tensor_scalar(x_sb, x_sb, mv[:, 0:1], rstd,
                                        op0=Alu.subtract, op1=Alu.mult)
                nc.vector.tensor_tensor(x_sb, x_sb, BC(f"ln_w_{l}"), Alu.mult)
                nc.vector.tensor_tensor(feat_nat, x_sb, BC(f"ln_b_{l}"), Alu.add)

                fT_ps = ps_m.tile([E, NL], F32, name=f"fT_ps_{l}", tag="m")
                nc.tensor.transpose(fT_ps, feat_nat, ident_f)
                nc.vector.tensor_copy(featT, fT_ps)

                # MLP (gelu via tanh approximation, sim-supported)
                o2_ps = ps_o.tile([NL, E], F32, name=f"o2_ps_{l}")
                for j in range(2):
                    m_ps = ps_m.tile([128, NL], F32, name=f"m_ps_{l}_{j}", tag="m")
                    nc.tensor.matmul(m_ps, W("w1T", l)[:, 128 * j:128 * (j + 1)],
                                     featT, start=True, stop=True)
                    xg = sb.tile([128, NL], F32, name=f"xg_{l}_{j}", tag="xg",
                                 bufs=2)
                    nc.scalar.activation(xg, m_ps, Act.Identity,
                                         bias=W("b1", l, j))
                    x2 = sb.tile([128, NL], F32, name=f"x2_{l}_{j}", tag="x2",
                                 bufs=2)
                    nc.vector.tensor_tensor(x2, xg, xg, Alu.mult)
                    nc.vector.tensor_tensor(x2, x2, xg, Alu.mult)
                    nc.vector.tensor_scalar(x2, x2, 0.044715, None, op0=Alu.mult)
                    nc.vector.tensor_tensor(x2, x2, xg, Alu.add)
                    tg = sb.tile([128, NL], F32, name=f"tg_{l}_{j}", tag="tg",
                                 bufs=2)
                    nc.scalar.activation(tg, x2, Act.Tanh,
                                         scale=0.7978845608028654)
                    mT = sb.tile([128, NL], F32, name=f"mT_{l}_{j}", bufs=3)
                    nc.vector.tensor_tensor(mT, xg, tg, Alu.mult)
                    nc.vector.tensor_tensor(mT, mT, xg, Alu.add)
                    nc.vector.tensor_scalar(mT, mT, 0.5, None, op0=Alu.mult)
                    nc.tensor.matmul(o2_ps, mT, W("w2T", l, j),
                                     start=(j == 0), stop=(j == 1))
                nc.vector.tensor_tensor(feat_nat, feat_nat, o2_ps, Alu.add)
                nc.vector.tensor_tensor(feat_nat, feat_nat, BC(f"mlp_b2_{l}"),
                                        Alu.add)

        # final featT
        with tc.tile_pool(name="ps_fin", bufs=2, space="PSUM") as ps_fin:
            fT_ps = ps_fin.tile([E, NL], F32, name="fT_ps", tag="fin")
            nc.tensor.transpose(fT_ps, feat_nat, ident_f)
            nc.vector.tensor_copy(featT, fT_ps)

            # ---------------- pair head prep (before seg/bot heads) ----------
            # T-layouts (signed-scaled): a2T = w2*aT ; b2sT = w2*(bT + b1)
            aT_ps = ps_fin.tile([HEAD, NL], F32, name="aT_ps", tag="fin")
            nc.tensor.matmul(aT_ps, W("WaT"), featT, start=True, stop=True)
            a2T_bf = persist.tile([HEAD, NL], BF16)
            nc.vector.tensor_scalar(a2T_bf, aT_ps, W("w2c"), None, op0=Alu.mult)
            bT_ps = ps_fin.tile([HEAD, NL], F32, name="bT_ps", tag="fin")
            nc.tensor.matmul(bT_ps, W("WbT"), featT, start=True, stop=True)
            b2sT_bf = persist.tile([HEAD, NL], BF16)
            nc.vector.tensor_scalar(b2sT_bf, bT_ps, W("b1c"), W("w2c"),
                                    op0=Alu.add, op1=Alu.mult)

            # natural layouts, scaled+stacked, then AG
            an_ps = ps_fin.tile([NL, HEAD], F32, name="an_ps", tag="fin")
            nc.tensor.matmul(an_ps, featT, W("WaT"), start=True, stop=True)
            bn_ps = ps_fin.tile([NL, HEAD], F32, name="bn_ps", tag="fin")
            nc.tensor.matmul(bn_ps, featT, W("WbT"), start=True, stop=True)
            ab_nat = persist.tile([NL, 2 * HEAD], F32)
            nc.vector.tensor_copy(ab_nat[:, 0:HEAD], an_ps)
            nc.vector.tensor_copy(ab_nat[:, HEAD:2 * HEAD], bn_ps)
            nc.vector.tensor_tensor(ab_nat, ab_nat, BC("baddrow"), Alu.add)
            nc.vector.tensor_tensor(ab_nat, ab_nat, BC("w2row"), Alu.mult)
            ab_bf = persist.tile([NL, 2 * HEAD], BF16)
            nc.vector.tensor_copy(ab_bf, ab_nat)

            ab_in = dram.tile([NL, 2 * HEAD], BF16)
            nc.sync.dma_start(ab_in, ab_bf)
            ab_out = dram.tile([N, 2 * HEAD], BF16, addr_space="Shared")
            nc.gpsimd.collective_compute(
                "AllGather", mybir.AluOpType.bypass,
                replica_groups=[list(range(NCORES))],
                ins=[ab_in.opt()], outs=[ab_out.opt()],
            )

            # slab source in DRAM: rows 0-63 a2T, 64-127 b2sT (bf16)
            slab_src = dram.tile([2 * HEAD, NL], BF16)
            nc.sync.dma_start(slab_src[0:HEAD, :], a2T_bf)
            nc.sync.dma_start(slab_src[HEAD:2 * HEAD, :], b2sT_bf)

            # combined slabs: a-set at cols [0, 8192), b-set at [8192, 16384)
            slabs_ab = persist.tile([128, 2 * HEAD * NL], BF16)
            for g in range(2 * HEAD // 8):
                nc.sync.dma_start(
                    slabs_ab[:, NL * 8 * g:NL * 8 * (g + 1)].rearrange(
                        "p (h f) -> p h f", h=8),
                    part_bcast(slab_src[8 * g:8 * (g + 1), :]))

            cols_bf = persist.tile([128, NCORES * 2 * HEAD], BF16)
            nc.sync.dma_start(cols_bf.rearrange("p (c f) -> p c f", c=NCORES),
                              ab_out.rearrange("(c r) f -> r c f", r=128))
            cols_f = persist.tile([128, NCORES * 2 * HEAD], F32)
            nc.vector.tensor_copy(cols_f, cols_bf)

            # ---------------- seg / bottom heads ----------------
            with tc.tile_pool(name="head_sb", bufs=2) as hsb:
                s1_ps = ps_fin.tile([HEAD, NL], F32, name="s1_ps", tag="fin")
                nc.tensor.matmul(s1_ps, W("sw1T"), featT, start=True, stop=True)
                s1T = hsb.tile([HEAD, NL], F32)
                nc.scalar.activation(s1T, s1_ps, Act.Relu, bias=W("sb1"))
                s2_ps = ps_fin.tile([NCLS, NL], F32, name="s2_ps", tag="fin")
                nc.tensor.matmul(s2_ps, W("sw2T"), s1T, start=True, stop=True)
                segT = hsb.tile([NCLS, NL], F32)
                nc.scalar.activation(segT, s2_ps, Act.Identity, bias=W("sb2"))
                nc.sync.dma_start(segT_out[:, :], segT)

                t1_ps = ps_fin.tile([HEAD, NL], F32, name="t1_ps", tag="fin")
                nc.tensor.matmul(t1_ps, W("bw1T"), featT, start=True, stop=True)
                t1T = hsb.tile([HEAD, NL], F32)
                nc.scalar.activation(t1T, t1_ps, Act.Relu, bias=W("bb1"))
                t2_ps = ps_fin.tile([1, NL], F32, name="t2_ps", tag="fin")
                nc.tensor.matmul(t2_ps, W("bw2T"), t1T, start=True, stop=True)
                botT = hsb.tile([1, NL], F32)
                nc.scalar.activation(botT, t2_ps, Act.Identity, bias=W("bb2"))
                nc.sync.dma_start(bot_out[:, :], botT)

        # ------- pair head main loop: R and M passes per chunk, wide ops -------
        # h-axis sorted (positive w2 first). Groups of 8 h-tiles:
        #   ACT groups: 8x [128,128] Relu ops (positive h only)
        #   DVE groups: wide TT-add (cols broadcast via 0-stride) + wide TS max/min
        # all accumulated by PE identity-matmuls into a [128, 512] PSUM (4 lanes),
        # folded at the end. Chunk result = mR*R + mM*M + mB.
        n_groups = HEAD // 8
        n_act_groups = min(N_ACT_GROUPS, npos // 8)
        with tc.tile_pool(name="pair_ps", bufs=3, space="PSUM") as pps, \
             tc.tile_pool(name="pair_sb", bufs=8) as psb, \
             tc.tile_pool(name="mask_sb", bufs=2) as msb, \
             tc.tile_pool(name="out_sb", bufs=2) as osb:

            def chunk_pass(jc, pas):
                soff = 0 if pas == "R" else HEAD * NL          # slab set
                coff = 128 * jc + (HEAD if pas == "R" else 0)  # col half
                acc_ps = pps.tile([128, 512], F32, name=f"accps_{jc}_{pas}",
                                  tag="accps")
                for g in range(n_groups):
                    wb = psb.tile([128, 8 * NL], BF16, name=f"wb_{jc}_{pas}_{g}",
                                  tag="wb", bufs=4)
                    if g < n_act_groups:
                        for u in range(8):
                            h = 8 * g + u
                            nc.scalar.activation(
                                wb[:, NL * u:NL * (u + 1)],
                                slabs_ab[:, soff + NL * h:soff + NL * (h + 1)],
                                Act.Relu,
                                bias=cols_f[:, coff + h:coff + h + 1])
                    else:
                        h0 = 8 * g
                        csl = cols_bf[:, coff + h0:coff + h0 + 8]
                        cb = bass.AP(tensor=csl.tensor, offset=csl.offset,
                                     ap=[list(csl.ap[0]), list(csl.ap[1]),
                                         [0, NL]])
                        nc.vector.tensor_tensor(
                            wb.rearrange("p (h f) -> p h f", h=8),
                            slabs_ab[:, soff + NL * h0:soff + NL * (h0 + 8)]
                            .rearrange("p (h f) -> p h f", h=8),
                            cb, Alu.add)
                        # relu: max for positive-w2 h's, min for negative
                        lo, hi = h0, h0 + 8
                        if hi <= npos or lo >= npos:
                            op1 = Alu.max if lo < npos else Alu.min
                            nc.vector.tensor_scalar(wb, wb, 0.0, None, op0=op1)
                        else:
                            kpos = npos - lo
                            nc.vector.tensor_scalar(
                                wb[:, :NL * kpos], wb[:, :NL * kpos], 0.0, None,
                                op0=Alu.max)
                            nc.vector.tensor_scalar(
                                wb[:, NL * kpos:], wb[:, NL * kpos:], 0.0, None,
                                op0=Alu.min)
                    for q in range(2):
                        nc.tensor.matmul(acc_ps, ident_b,
                                         wb[:, 512 * q:512 * (q + 1)],
                                         start=(g == 0 and q == 0),
                                         stop=(g == n_groups - 1 and q == 1))
                cp = psb.tile([128, 512], F32, name=f"cp_{jc}_{pas}", tag="cp",
                              bufs=3)
                nc.scalar.copy(cp, acc_ps)
                f1 = psb.tile([128, 256], F32, name=f"f1_{jc}_{pas}", tag="f1",
                              bufs=3)
                nc.vector.tensor_tensor(f1, cp[:, 0:256], cp[:, 256:512], Alu.add)
                tot = psb.tile([128, NL], F32, name=f"tot_{jc}_{pas}", tag="tot",
                               bufs=3)
                nc.vector.tensor_tensor(tot, f1[:, 0:128], f1[:, 128:256], Alu.add)
                return tot

            for jc in range(NCORES):
                totR = chunk_pass(jc, "R")
                totM = chunk_pass(jc, "M")
                m3 = msb.tile([128, 3 * NL], F32, name=f"m3_{jc}", tag="m3")
                nc.sync.dma_start(m3, P["masks3"][128 * jc:128 * (jc + 1), :])
                t1 = osb.tile([128, NL], F32, name=f"t1_{jc}", tag="t1")
                nc.vector.tensor_tensor(t1, totR, m3[:, 0:NL], Alu.mult)
                t2 = osb.tile([128, NL], F32, name=f"t2_{jc}", tag="t2")
                nc.vector.tensor_tensor(t2, totM, m3[:, NL:2 * NL], Alu.mult)
                nc.vector.tensor_tensor(t1, t1, t2, Alu.add)
                nc.vector.tensor_tensor(t1, t1, m3[:, 2 * NL:3 * NL], Alu.add)
                nc.sync.dma_start(instT_out[128 * jc:128 * (jc + 1), :], t1)

    nc.compile()
    return nc


def _blob_spec():
    """Column layout of the packed parameter blob: key -> (partitions, c0, c1)."""
    spec = {}
    col = 0

    def add(key, p, f):
        nonlocal col
        spec[key] = (p, col, col + f)
        col += f

    add("xT", NODE_ATTR + GRID, NL)
    add("emb_wT", NODE_ATTR + GRID, E)
    add("emb_b", E, 1)
    for l in range(L):
        add(f"ipT_{l}", E, 704)
        for j in range(6):
            add(f"ipb_{l}_{j}", 96, 1)
        add(f"ipb_{l}_6", 128, 1)
        add(f"owT_{l}", E, E)
        add(f"w1T_{l}", E, MLP_H)
        for j in range(2):
            add(f"b1_{l}_{j}", 128, 1)
        for j in range(2):
            add(f"w2T_{l}_{j}", 128, E)
    add("sw1T", E, HEAD)
    add("sb1", HEAD, 1)
    add("sw2T", HEAD, NCLS)
    add("sb2", NCLS, 1)
    add("bw1T", E, HEAD)
    add("bb1", HEAD, 1)
    add("bw2T", HEAD, 1)
    add("bb2", 1, 1)
    add("WaT", E, HEAD)
    add("WbT", E, HEAD)
    add("w2c", HEAD, 1)
    add("b1c", HEAD, 1)
    return col, spec


def _host_prep(inputs):
    f = np.float32
    x = np.asarray(inputs["x"], f)
    node_w = np.asarray(inputs["node_w"], f)
    node_b = np.asarray(inputs["node_b"], f)
    grid_w = np.asarray(inputs["grid_w"], f)
    grid_b = np.asarray(inputs["grid_b"], f)

    emb_wT = np.zeros((NODE_ATTR + GRID, E), f)
    emb_wT[:NODE_ATTR, :64] = node_w.T
    emb_wT[NODE_ATTR:, 64:] = grid_w.T
    emb_b = np.concatenate([node_b, grid_b])[:, None].astype(f)

    # head-padded q/k: head h -> rows 96*(h//3) + 32*(h%3) .. +16
    ipw = np.asarray(inputs["in_proj_w"], f)
    ipb = np.asarray(inputs["in_proj_b"], f)
    qs = f(1.0 / np.sqrt(HD))
    qpad = np.zeros((L, 288, E), f)
    kpad = np.zeros((L, 288, E), f)
    bq = np.zeros((L, 288), f)
    bk = np.zeros((L, 288), f)
    for h in range(NH):
        dst = slice(96 * (h // 3) + 32 * (h % 3), 96 * (h // 3) + 32 * (h % 3) + HD)
        qpad[:, dst, :] = ipw[:, HD * h:HD * (h + 1), :] * qs
        bq[:, dst] = ipb[:, HD * h:HD * (h + 1)] * qs
        kpad[:, dst, :] = ipw[:, E + HD * h:E + HD * (h + 1), :]
        bk[:, dst] = ipb[:, E + HD * h:E + HD * (h + 1)]
    ipT = np.concatenate([qpad.transpose(0, 2, 1), kpad.transpose(0, 2, 1),
                          ipw[:, 2 * E:, :].transpose(0, 2, 1)], axis=2)
    ipb_pad = np.concatenate([bq, bk, ipb[:, 2 * E:]], axis=1)  # [L, 704]

    # pair head: sort h-axis by sign of w2 (positive first)
    inst_w1 = np.asarray(inputs["inst_w1"], f)
    w2 = np.asarray(inputs["inst_w2"], f)[0]
    b1i = np.asarray(inputs["inst_b1"], f)
    b2i = float(np.asarray(inputs["inst_b2"], f)[0])
    perm = np.argsort((w2 <= 0).astype(np.int32), kind="stable")
    npos = int((w2 > 0).sum())
    w2_s = w2[perm]
    b1_s = b1i[perm]
    WaT = np.ascontiguousarray(inst_w1[perm][:, :E].T)
    WbT = np.ascontiguousarray(inst_w1[perm][:, E:].T)

    blob_cols, spec = _blob_spec()
    blob = np.zeros((128, blob_cols), f)

    def put(key, arr):
        p, c0, c1 = spec[key]
        blob[:p, c0:c1] = np.asarray(arr, f).reshape(p, c1 - c0)

    put("emb_wT", emb_wT)
    put("emb_b", emb_b)
    for l in range(L):
        put(f"ipT_{l}", ipT[l])
        for j in range(6):
            put(f"ipb_{l}_{j}", ipb_pad[l, 96 * j:96 * (j + 1), None])
        put(f"ipb_{l}_6", ipb_pad[l, 576:704, None])
        put(f"owT_{l}", np.asarray(inputs["out_w"], f)[l].T)
        put(f"w1T_{l}", np.asarray(inputs["mlp_w1"], f)[l].T)
        for j in range(2):
            put(f"b1_{l}_{j}",
                np.asarray(inputs["mlp_b1"], f)[l, 128 * j:128 * (j + 1), None])
            put(f"w2T_{l}_{j}",
                np.asarray(inputs["mlp_w2"], f)[l].T[128 * j:128 * (j + 1), :])
    put("sw1T", np.asarray(inputs["seg_w1"], f).T)
    put("sb1", np.asarray(inputs["seg_b1"], f)[:, None])
    put("sw2T", np.asarray(inputs["seg_w2"], f).T)
    put("sb2", np.asarray(inputs["seg_b2"], f)[:, None])
    put("bw1T", np.asarray(inputs["bot_w1"], f).T)
    put("bb1", np.asarray(inputs["bot_b1"], f)[:, None])
    put("bw2T", np.asarray(inputs["bot_w2"], f).T)
    put("bb2", np.asarray(inputs["bot_b2"], f).reshape(1, 1))
    put("WaT", WaT)
    put("WbT", WbT)
    put("w2c", w2_s[:, None])
    put("b1c", b1_s[:, None])

    rows14 = np.zeros((14, E), f)
    rows14[0:3] = np.asarray(inputs["out_b"], f)
    rows14[3:6] = np.asarray(inputs["ln_w"], f)
    rows14[6:9] = np.asarray(inputs["ln_b"], f)
    rows14[9:12] = np.asarray(inputs["mlp_b2"], f)
    rows14[12] = np.concatenate([w2_s, w2_s])
    rows14[13] = np.concatenate([np.zeros(HEAD, f), b1_s])

    gj = np.arange(N)[:, None]
    in_maps = []
    for c in range(NCORES):
        b = blob.copy()
        p, c0, c1 = spec["xT"]
        b[:p, c0:c1] = x[NL * c:NL * (c + 1), :].T

        # masks: mR selects M[gi, j] (j > gi), mM selects M[j, gi] (j < gi)
        gi = (NL * c + np.arange(NL))[None, :]
        mR = (gj > gi).astype(f)
        mM = (gj < gi).astype(f)
        m3 = np.concatenate([mR, mM, b2i * (mR + mM)], axis=1)

        in_maps.append({"blob": b, "rows14": rows14, "masks3": m3})

    return in_maps, npos


def _get_graph(npos):
    if npos not in _GRAPH_CACHE:
        _GRAPH_CACHE[npos] = _build(npos)
    return _GRAPH_CACHE[npos]


def run_on_hw(inputs, trace=False):
    import sys
    if "/opt/trn_rl_repo" not in sys.path:
        sys.path.insert(0, "/opt/trn_rl_repo")
    from concourse.bass_utils import run_bass_kernel_spmd
    in_maps, npos = _host_prep(inputs)
    nc = _get_graph(npos)
    return run_bass_kernel_spmd(nc, in_maps, core_ids=list(range(NCORES)),
                                trace=trace)


def _assemble(results):
    f = np.float32
    seg = np.concatenate([np.asarray(r["segT_out"], f).T for r in results], axis=0)
    bottom = np.concatenate([np.asarray(r["bot_out"], f)[0] for r in results])
    inst = np.empty((N, N), f)
    for c, r in enumerate(results):
        inst[:, NL * c:NL * (c + 1)] = np.asarray(r["instT_out"], f)
    return seg, inst, bottom


def kernel(**inputs):
    res = run_on_hw(inputs, trace=False)
    return _assemble(res.results)
